# revision 19
# baseline (speedup 1.0000x reference)
"""CircleLoss Trainium2 kernel.

Full-input contract: kernel(mat, pos_mask, neg_mask) -> loss [256] f32.

Math: with block masks (cols [0,32768) positive, [32768,65536) negative)
and mat values in [-0.25, 1.25] (setup uses uniform [0,1)), the relu
terms in CircleLoss are affine:
    sp = -G*relu(OP-x)*(x-DP) = 16(x-1)^2 - 1
    sn =  G*relu(x-ON)*(x-DN) = 16 x^2    - 1
loss[b] = log1p( sum_pos exp(sp) * sum_neg exp(sn) )

Sharding: data-parallel over B=256 rows -> 32 rows per core on 8 cores.
The 8 MiB/core input stream is the roofline; data is cast to bf16
during the DMA (SWDGE) so the DVE squares run in 2x perf mode and never
pace the pipeline.  Device emits per-partition partial sums of
exp(16*x^2) (neg) / exp(16*x*(x-2)) (pos); the tiny final fold
(block-sum, product, log1p) runs on the host.
"""

import os
from contextlib import ExitStack

import numpy as np

B = 256
NCOLS = 65536
NPOS = 32768
N_CORES = 8
R = B // N_CORES  # 32 rows per core
GAMMA = 16.0
MARGIN = 0.25
OP, ON = 1.0 + MARGIN, -MARGIN
DP, DN = 1.0 - MARGIN, MARGIN

BLK = 4  # row-blocks per half; partition p = 4*row + blk
HALF_FREE = NPOS // BLK  # 8192 elements per partition per half
# uniform DMA chunks (2048 = 8KB f32 read descriptors, fewest dispatches);
# exp groups coincide with chunks
CSIZES = (2048, 2048, 2048, 2048)
assert sum(CSIZES) == HALF_FREE
NCH = len(CSIZES)
GSIZES = (2048, 2048, 2048, 2048)  # exp-group free sizes (1 chunk each)
assert sum(GSIZES) == HALF_FREE
NG = len(GSIZES)  # exp groups per half

LAST = None  # BassKernelResults of the most recent device run (for test.py)

_prog_cache = {}


def _patch_act_tables():
    """Restrict Exp/Square/Ln to the natural_log_exp_and_others set so the
    whole kernel needs a single ACT_TABLE_LOAD.  Set ids (list positions)
    are preserved; only membership is pruned."""
    import concourse.bacc as bacc_mod
    import concourse.mybir as mybir

    if getattr(bacc_mod, "_circle_tables_patched", False):
        return
    orig = bacc_mod.get_activation_tables
    ours = {
        mybir.ActivationFunctionType.Exp,
        mybir.ActivationFunctionType.Square,
        mybir.ActivationFunctionType.Ln,
    }

    def patched(arch):
        tabs = orig(arch)
        return {
            name: (fns if name == "natural_log_exp_and_others" else fns - ours)
            for name, fns in tabs.items()
        }

    bacc_mod.get_activation_tables = patched
    bacc_mod._circle_tables_patched = True


def _patch_slim_teardown():
    """Skip the tile-context epilogue semaphore recycling (gpsimd dma_reset +
    sem_clear + trailing all-engine barrier).  That hygiene only matters when
    the same loaded NEFF executes again reusing the semaphore window; this
    kernel's NEFF is built and executed once per process.  Keeps the drain
    (with semaphore waits on all tracked work) and the first barrier, so the
    output DMA is complete before the NEFF retires."""
    import concourse.tile as tile_mod
    from concourse.vector_clock import ScopedClock

    if getattr(tile_mod, "_circle_slim_teardown", False):
        return

    def _slim_drain_and_barrier(self, tick_clock, wait_clock):
        del tick_clock, wait_clock  # per-engine drains in the barrier below
        # already wait for all engine-local work and DGE queue completion
        self.nc.all_engine_barrier()
        popped = self.nc._tile_sem_poison_stack.pop()
        assert popped is self._sem_poison

    tile_mod.TileContext._drain_and_barrier = _slim_drain_and_barrier
    tile_mod._circle_slim_teardown = True


def _build_program():
    import concourse.mybir as mybir
    from concourse.bacc import Bacc
    from concourse.tile import TileContext

    f32 = mybir.dt.float32
    bf16 = mybir.dt.bfloat16
    Exp = mybir.ActivationFunctionType.Exp

    _patch_act_tables()
    _patch_slim_teardown()
    nc = Bacc()
    mat = nc.dram_tensor("mat", [R, NCOLS], f32, kind="ExternalInput")
    out = nc.dram_tensor("out", [128, 2 * NG], f32, kind="ExternalOutput")

    # [32, 4, 8192] views: partition p = 4*row + blk, free = within-block col
    pos = mat[:, 0:NPOS].rearrange("r (b f) -> r b f", b=BLK)
    neg = mat[:, NPOS:NCOLS].rearrange("r (b f) -> r b f", b=BLK)

    with TileContext(nc) as tc, ExitStack() as ctx:
        # all chunk tiles stay live -> no buffer-reuse stalls anywhere
        pool = ctx.enter_context(tc.tile_pool(name="data", bufs=2 * NCH))
        sqpool = ctx.enter_context(tc.tile_pool(name="sq", bufs=2 * NG))
        epool = ctx.enter_context(tc.tile_pool(name="e", bufs=3))
        spool = ctx.enter_context(tc.tile_pool(name="stats", bufs=1))

        stats = spool.tile([128, 2 * NG], f32)

        # Device computes shifted sums (squares are single DVE ops):
        #   neg: sum exp(16*x^2)          = e^16 * sum exp(sn+1)
        #   pos: sum exp(16*(x^2-2x))     = e^-16 * sum exp(16(1-x)^2)
        # via (1-x)^2 - 1 = x*(x-2).  Host folds the net e^14 factor.
        sq_tiles = {}
        for g in range(NG):
            for half in (0, 1):  # 0 = neg, 1 = pos
                sq_tiles[(g, half)] = sqpool.tile(
                    [128, GSIZES[g]], bf16, tag="sq", name=f"sq_{g}_{half}"
                )

        goff = [sum(GSIZES[:g]) for g in range(NG)]
        off = 0
        for c, FC in enumerate(CSIZES):
            g = max(i for i in range(NG) if goff[i] <= off)
            sub = off - goff[g]
            for half in (0, 1):  # 0 = neg, 1 = pos
                src = neg if half == 0 else pos
                t = pool.tile([128, FC], bf16, tag="data")
                # SWDGE (gpsimd) casts f32->bf16 in flight; bf16 tiles give
                # the neg tensor_tensor square DVE 2x mode and halve SBUF
                nc.gpsimd.dma_start(out=t[:], in_=src[:, :, off : off + FC])
                if half == 0:
                    # x^2 (2x perf mode)
                    nc.vector.tensor_tensor(
                        sq_tiles[(g, 0)][:, sub : sub + FC],
                        t[:], t[:], mybir.AluOpType.mult,
                    )
                else:
                    # x*(x-2) = (1-x)^2 - 1
                    nc.vector.scalar_tensor_tensor(
                        sq_tiles[(g, 1)][:, sub : sub + FC],
                        t[:], -2.0, t[:],
                        mybir.AluOpType.add, mybir.AluOpType.mult,
                    )
            off += FC
            if off - goff[g] == GSIZES[g]:
                for half in (0, 1):
                    col = half * NG + g  # cols [0,NG) neg, [NG,2NG) pos
                    e = epool.tile([128, GSIZES[g]], bf16, tag="e")
                    nc.scalar.activation(
                        e[:], sq_tiles[(g, half)][:], Exp,
                        bias=0.0, scale=GAMMA,
                        accum_out=stats[:, col : col + 1],
                    )

        nc.sync.dma_start(out=out[:, :], in_=stats[:])

    nc.finalize()
    return nc


def _host_reference(mat, pos_mask, neg_mask):
    """General fallback for inputs that don't match the expected structure."""
    x = mat.astype(np.float64)
    sp = -GAMMA * np.maximum(OP - x, 0.0) * (x - DP)
    sn = GAMMA * np.maximum(x - ON, 0.0) * (x - DN)
    psum = (np.exp(sp) * (pos_mask == 1)).sum(axis=1)
    nsum = (np.exp(sn) * (neg_mask == 1)).sum(axis=1)
    return np.log1p(psum * nsum).astype(np.float32)


def _structured(mat, pos_mask, neg_mask):
    if mat.shape != (B, NCOLS):
        return False
    if mat.min() < -MARGIN or mat.max() > OP:
        return False
    if not (pos_mask[:, :NPOS] == 1).all() or (pos_mask[:, NPOS:] == 1).any():
        return False
    if not (neg_mask[:, NPOS:] == 1).all() or (neg_mask[:, :NPOS] == 1).any():
        return False
    return True


def kernel(mat, pos_mask, neg_mask):
    global LAST
    mat = np.ascontiguousarray(mat, dtype=np.float32)
    if not _structured(mat, pos_mask, neg_mask):
        return _host_reference(mat, pos_mask, neg_mask)

    from concourse.bass_utils import run_bass_kernel_spmd

    if "prog" not in _prog_cache:
        _prog_cache["prog"] = _build_program()
    nc = _prog_cache["prog"]

    in_maps = [
        {"mat": np.ascontiguousarray(mat[i * R : (i + 1) * R])}
        for i in range(N_CORES)
    ]
    kwargs = {}
    if os.environ.get("BASS_TRACE"):
        kwargs["trace"] = True
        td = os.environ.get("KERNEL_TRACE_DIR")
        if td:
            os.makedirs(td, exist_ok=True)
            kwargs["tmpdir"] = td
    res = run_bass_kernel_spmd(nc, in_maps, core_ids=list(range(N_CORES)), **kwargs)
    LAST = res
    parts = []
    for i in range(N_CORES):
        stats = res.results[i]["out"].astype(np.float64)  # [128, 2*NG]
        s = stats.reshape(R, BLK, 2 * NG)  # p = 4*row + blk
        nsum = s[:, :, 0:NG].sum(axis=(1, 2))  # sum exp(16 x^2)      [R]
        psum = s[:, :, NG : 2 * NG].sum(axis=(1, 2))  # sum exp(16 x(x-2))
        parts.append(np.log1p(np.exp(14.0) * psum * nsum))
    return np.concatenate(parts).astype(np.float32)


# revision 20
# speedup vs baseline: 1.0611x; 1.0611x over previous
"""CircleLoss Trainium2 kernel.

Full-input contract: kernel(mat, pos_mask, neg_mask) -> loss [256] f32.

Math: with block masks (cols [0,32768) positive, [32768,65536) negative)
and mat values in [-0.25, 1.25] (setup uses uniform [0,1)), the relu
terms in CircleLoss are affine:
    sp = -G*relu(OP-x)*(x-DP) = 16(x-1)^2 - 1
    sn =  G*relu(x-ON)*(x-DN) = 16 x^2    - 1
loss[b] = log1p( sum_pos exp(sp) * sum_neg exp(sn) )

Sharding: data-parallel over B=256 rows -> 32 rows per core on 8 cores.
The 8 MiB/core input stream is the roofline; data is cast to bf16
during the DMA (SWDGE) so the DVE squares run in 2x perf mode and never
pace the pipeline.  Device emits per-partition partial sums of
exp(16*x^2) (neg) / exp(16*x*(x-2)) (pos); the tiny final fold
(block-sum, product, log1p) runs on the host.
"""

import os
from contextlib import ExitStack

import numpy as np

B = 256
NCOLS = 65536
NPOS = 32768
N_CORES = 8
R = B // N_CORES  # 32 rows per core
GAMMA = 16.0
MARGIN = 0.25
OP, ON = 1.0 + MARGIN, -MARGIN
DP, DN = 1.0 - MARGIN, MARGIN

BLK = 4  # row-blocks per half; partition p = 4*row + blk
HALF_FREE = NPOS // BLK  # 8192 elements per partition per half
# mostly-big DMA chunks (8KB f32 read descriptors amortize best) with a
# tapered tail so the post-stream square->exp drain is short; exp groups
# coincide with chunks
CSIZES = (2048, 2048, 2048, 1536, 512)
assert sum(CSIZES) == HALF_FREE
NCH = len(CSIZES)
GSIZES = (2048, 2048, 2048, 1536, 512)  # exp-group free sizes (1 chunk each)
assert sum(GSIZES) == HALF_FREE
NG = len(GSIZES)  # exp groups per half

LAST = None  # BassKernelResults of the most recent device run (for test.py)

_prog_cache = {}


def _patch_act_tables():
    """Restrict Exp/Square/Ln to the natural_log_exp_and_others set so the
    whole kernel needs a single ACT_TABLE_LOAD.  Set ids (list positions)
    are preserved; only membership is pruned."""
    import concourse.bacc as bacc_mod
    import concourse.mybir as mybir

    if getattr(bacc_mod, "_circle_tables_patched", False):
        return
    orig = bacc_mod.get_activation_tables
    ours = {
        mybir.ActivationFunctionType.Exp,
        mybir.ActivationFunctionType.Square,
        mybir.ActivationFunctionType.Ln,
    }

    def patched(arch):
        tabs = orig(arch)
        return {
            name: (fns if name == "natural_log_exp_and_others" else fns - ours)
            for name, fns in tabs.items()
        }

    bacc_mod.get_activation_tables = patched
    bacc_mod._circle_tables_patched = True


def _patch_slim_teardown():
    """Skip the tile-context epilogue semaphore recycling (gpsimd dma_reset +
    sem_clear + trailing all-engine barrier).  That hygiene only matters when
    the same loaded NEFF executes again reusing the semaphore window; this
    kernel's NEFF is built and executed once per process.  Keeps the drain
    (with semaphore waits on all tracked work) and the first barrier, so the
    output DMA is complete before the NEFF retires."""
    import concourse.tile as tile_mod
    from concourse.vector_clock import ScopedClock

    if getattr(tile_mod, "_circle_slim_teardown", False):
        return

    def _slim_drain_and_barrier(self, tick_clock, wait_clock):
        del tick_clock, wait_clock  # per-engine drains in the barrier below
        # already wait for all engine-local work and DGE queue completion
        self.nc.all_engine_barrier()
        popped = self.nc._tile_sem_poison_stack.pop()
        assert popped is self._sem_poison

    tile_mod.TileContext._drain_and_barrier = _slim_drain_and_barrier
    tile_mod._circle_slim_teardown = True


def _build_program():
    import concourse.mybir as mybir
    from concourse.bacc import Bacc
    from concourse.tile import TileContext

    f32 = mybir.dt.float32
    bf16 = mybir.dt.bfloat16
    Exp = mybir.ActivationFunctionType.Exp

    _patch_act_tables()
    _patch_slim_teardown()
    nc = Bacc()
    mat = nc.dram_tensor("mat", [R, NCOLS], f32, kind="ExternalInput")
    out = nc.dram_tensor("out", [128, 2 * NG], f32, kind="ExternalOutput")

    # [32, 4, 8192] views: partition p = 4*row + blk, free = within-block col
    pos = mat[:, 0:NPOS].rearrange("r (b f) -> r b f", b=BLK)
    neg = mat[:, NPOS:NCOLS].rearrange("r (b f) -> r b f", b=BLK)

    with TileContext(nc) as tc, ExitStack() as ctx:
        # all chunk tiles stay live -> no buffer-reuse stalls anywhere
        pool = ctx.enter_context(tc.tile_pool(name="data", bufs=2 * NCH))
        sqpool = ctx.enter_context(tc.tile_pool(name="sq", bufs=2 * NG))
        epool = ctx.enter_context(tc.tile_pool(name="e", bufs=3))
        spool = ctx.enter_context(tc.tile_pool(name="stats", bufs=1))

        stats = spool.tile([128, 2 * NG], f32)

        # Device computes shifted sums (squares are single DVE ops):
        #   neg: sum exp(16*x^2)          = e^16 * sum exp(sn+1)
        #   pos: sum exp(16*(x^2-2x))     = e^-16 * sum exp(16(1-x)^2)
        # via (1-x)^2 - 1 = x*(x-2).  Host folds the net e^14 factor.
        sq_tiles = {}
        for g in range(NG):
            for half in (0, 1):  # 0 = neg, 1 = pos
                sq_tiles[(g, half)] = sqpool.tile(
                    [128, GSIZES[g]], bf16, tag="sq", name=f"sq_{g}_{half}"
                )

        goff = [sum(GSIZES[:g]) for g in range(NG)]
        off = 0
        for c, FC in enumerate(CSIZES):
            g = max(i for i in range(NG) if goff[i] <= off)
            sub = off - goff[g]
            for half in (0, 1):  # 0 = neg, 1 = pos
                src = neg if half == 0 else pos
                t = pool.tile([128, FC], bf16, tag="data")
                # SWDGE (gpsimd) casts f32->bf16 in flight; bf16 tiles give
                # the neg tensor_tensor square DVE 2x mode and halve SBUF
                nc.gpsimd.dma_start(out=t[:], in_=src[:, :, off : off + FC])
                if half == 0:
                    # x^2 (2x perf mode)
                    nc.vector.tensor_tensor(
                        sq_tiles[(g, 0)][:, sub : sub + FC],
                        t[:], t[:], mybir.AluOpType.mult,
                    )
                else:
                    # x*(x-2) = (1-x)^2 - 1
                    nc.vector.scalar_tensor_tensor(
                        sq_tiles[(g, 1)][:, sub : sub + FC],
                        t[:], -2.0, t[:],
                        mybir.AluOpType.add, mybir.AluOpType.mult,
                    )
            off += FC
            if off - goff[g] == GSIZES[g]:
                for half in (0, 1):
                    col = half * NG + g  # cols [0,NG) neg, [NG,2NG) pos
                    e = epool.tile([128, GSIZES[g]], bf16, tag="e")
                    nc.scalar.activation(
                        e[:], sq_tiles[(g, half)][:], Exp,
                        bias=0.0, scale=GAMMA,
                        accum_out=stats[:, col : col + 1],
                    )

        nc.sync.dma_start(out=out[:, :], in_=stats[:])

    nc.finalize()
    return nc


def _host_reference(mat, pos_mask, neg_mask):
    """General fallback for inputs that don't match the expected structure."""
    x = mat.astype(np.float64)
    sp = -GAMMA * np.maximum(OP - x, 0.0) * (x - DP)
    sn = GAMMA * np.maximum(x - ON, 0.0) * (x - DN)
    psum = (np.exp(sp) * (pos_mask == 1)).sum(axis=1)
    nsum = (np.exp(sn) * (neg_mask == 1)).sum(axis=1)
    return np.log1p(psum * nsum).astype(np.float32)


def _structured(mat, pos_mask, neg_mask):
    if mat.shape != (B, NCOLS):
        return False
    if mat.min() < -MARGIN or mat.max() > OP:
        return False
    if not (pos_mask[:, :NPOS] == 1).all() or (pos_mask[:, NPOS:] == 1).any():
        return False
    if not (neg_mask[:, NPOS:] == 1).all() or (neg_mask[:, :NPOS] == 1).any():
        return False
    return True


def kernel(mat, pos_mask, neg_mask):
    global LAST
    mat = np.ascontiguousarray(mat, dtype=np.float32)
    if not _structured(mat, pos_mask, neg_mask):
        return _host_reference(mat, pos_mask, neg_mask)

    from concourse.bass_utils import run_bass_kernel_spmd

    if "prog" not in _prog_cache:
        _prog_cache["prog"] = _build_program()
    nc = _prog_cache["prog"]

    in_maps = [
        {"mat": np.ascontiguousarray(mat[i * R : (i + 1) * R])}
        for i in range(N_CORES)
    ]
    kwargs = {}
    if os.environ.get("BASS_TRACE"):
        kwargs["trace"] = True
        td = os.environ.get("KERNEL_TRACE_DIR")
        if td:
            os.makedirs(td, exist_ok=True)
            kwargs["tmpdir"] = td
    res = run_bass_kernel_spmd(nc, in_maps, core_ids=list(range(N_CORES)), **kwargs)
    LAST = res
    parts = []
    for i in range(N_CORES):
        stats = res.results[i]["out"].astype(np.float64)  # [128, 2*NG]
        s = stats.reshape(R, BLK, 2 * NG)  # p = 4*row + blk
        nsum = s[:, :, 0:NG].sum(axis=(1, 2))  # sum exp(16 x^2)      [R]
        psum = s[:, :, NG : 2 * NG].sum(axis=(1, 2))  # sum exp(16 x(x-2))
        parts.append(np.log1p(np.exp(14.0) * psum * nsum))
    return np.concatenate(parts).astype(np.float32)
